# revision 30
# baseline (speedup 1.0000x reference)
"""AttnBlock (GroupNorm + spatial self-attention + residual) on 8 trn2 NeuronCores.

Sharding: 8 cores = 2 batches x 4 query-chunks of 1024 spatial positions.
Each core receives x[b] rolled so its query range is columns [0, 1024); all
cores run one identical SPMD program.

Host-side algebra (exact up to dropped softmax-invariant terms):
  scores^T[j,i] = hn[:,j] . (Wqk hn[:,i] + bqk)   with Wqk = C^-1/2 wk^T wq,
    bqk = C^-1/2 wk^T bq  (the bk term is constant over j -> softmax-invariant)
  out = x + sum_j softmax_j . (Wov hn[:,j]) + bov  with Wov = wo wv,
    bov = wo bv + bo      (softmax rows sum to 1 -> bias moves outside)

Device-side GroupNorm folding: hn = A.x + B per channel; the A-scales fold
into runtime-quantized fp8 weights, B-terms into runtime biases. All five
big matmul groups run fp8e4m3 DoubleRow (2 MACs/PE/cycle). exp(s-5) keeps
es in fp8 range (softmax-invariant shift). Residual stays f32 via xf.

Startup-latency engineering: partition-major DRAM layouts give 4-16KB DMA
packets; groupnorm stats are split per-tile between ACT (first half,
Copy/Square accum) and DVE (second half, bn_stats), combined in the group
reduction matmul with a half-weighted second accumulation; rstd uses a
Newton rsqrt on DVE int/float ALU so the only ACT functions ever used are
{Copy, Square, Identity, Exp} = one table set, preloaded during DMA wait.
"""

import ml_dtypes
import numpy as np

import concourse.bass as bass
import concourse.tile as tile
from concourse import bacc, mybir
from concourse import bass_utils

F32 = mybir.dt.float32
F32R = mybir.dt.float32r
BF16 = mybir.dt.bfloat16
FP8 = mybir.dt.float8e4
I32 = mybir.dt.int32
DR = mybir.MatmulPerfMode.DoubleRow
AOP = mybir.AluOpType

NPF8 = ml_dtypes.float8_e4m3fn

B, C, D, H, W = 2, 512, 4, 32, 32
L = D * H * W            # 4096
G = 32                   # groupnorm groups
EPS = 1e-6
P = 128
NT = C // P              # 4 channel tiles
NJ = L // P              # 32 key tiles
IC = 512                 # i-chunk width
LQ = 1024                # query cols per core
NIC = LQ // IC           # 2 i-chunks
NCORES = 8
NPAIR = NJ // 2          # 16 key-tile pairs per i-chunk
PD = 3                   # attention software-pipeline depth (es pairs ahead)

LH = 1536                # stats split: ACT does cols [0,LH), DVE does [LH,L)
DCH = [512, 512, 512, 512, 512]   # DVE bn_stats chunk widths (sum = L-LH)

QK_SCALE = 256.0         # host prescale on Wqk (fp8 subnormal avoidance)
OV_SCALE = 1.0           # no Wov prescale -> softmax denom needs no unfold
ES_SHIFT = -5.0          # softmax-invariant logit shift (max logit ~9.2)

_CACHE = {}


def _build():
    nc = bacc.Bacc(trn_type="TRN2", target_bir_lowering=False, debug=False,
                   num_devices=NCORES)
    x_d = nc.dram_tensor("x8", [NT, P, L], FP8, kind="ExternalInput").ap()
    xf_d = nc.dram_tensor("xf", [P, NT * LQ], BF16, kind="ExternalInput").ap()
    wqk_d = nc.dram_tensor("wqkT", [P, NT * C], FP8, kind="ExternalInput").ap()
    wov_d = nc.dram_tensor("wovT", [P, NT * C], FP8, kind="ExternalInput").ap()
    pg_d = nc.dram_tensor("pg", [P, NT * G], F32, kind="ExternalInput").ap()
    pgh_d = nc.dram_tensor("pgh", [P, NT * G], F32, kind="ExternalInput").ap()
    sel_d = nc.dram_tensor("sel", [G, C], F32R, kind="ExternalInput").ap()
    nwt_d = nc.dram_tensor("nwt", [G, 4], F32, kind="ExternalInput").ap()
    onesF_d = nc.dram_tensor("onesF", [P, P], F32R, kind="ExternalInput").ap()
    gamma_d = nc.dram_tensor("gamma", [P, NT], F32, kind="ExternalInput").ap()
    wg_d = nc.dram_tensor("wgT", [G, C], F32R, kind="ExternalInput").ap()
    vg_d = nc.dram_tensor("vgT", [G, C], F32R, kind="ExternalInput").ap()
    hqk_d = nc.dram_tensor("hqk", [P, NT], F32, kind="ExternalInput").ap()
    hov_d = nc.dram_tensor("hov", [P, NT], F32, kind="ExternalInput").ap()
    out_d = nc.dram_tensor("out", [C, LQ], BF16, kind="ExternalOutput").ap()

    with tile.TileContext(nc) as tc:
        with (
            tc.tile_pool(name="big", bufs=1) as big,
            tc.tile_pool(name="wp", bufs=1) as wp,
            tc.tile_pool(name="small", bufs=1) as small,
            tc.tile_pool(name="est", bufs=PD + 4) as est,
            tc.tile_pool(name="osb", bufs=3) as osb,
            tc.tile_pool(name="obf", bufs=4) as obf,
            tc.tile_pool(name="accp", bufs=4) as accp,
            tc.tile_pool(name="zp", bufs=8) as zp,
            tc.tile_pool(name="tmp", bufs=4) as tmp,
            tc.tile_pool(name="ps", bufs=3, space="PSUM") as ps,
            tc.tile_pool(name="pho", bufs=4, space="PSUM") as pho,
            tc.tile_pool(name="psum1", bufs=1, space="PSUM") as psum1,
        ):
            # ---- DMA: x8 tiles split across the two HWDGE rings
            # (partition-major: 4KB rows per tile -> big packets) ----
            xt8 = big.tile([P, NT, L], FP8, tag="xt8")
            for t in (0, 1):
                nc.sync.dma_start(xt8[:, t, :], x_d[t])
            pg = small.tile([P, NT, G], F32, tag="pg")
            nc.scalar.dma_start(pg[:], pg_d)
            pgh = small.tile([P, NT, G], F32, tag="pgh")
            nc.scalar.dma_start(pgh[:], pgh_d)
            sel = small.tile([G, NT, P], F32R, tag="sel")
            nc.scalar.dma_start(sel[:], sel_d.rearrange("g (t p) -> g t p", p=P))
            nwt = small.tile([G, 4], F32, tag="nwt")
            nc.scalar.dma_start(nwt[:], nwt_d)
            gam = small.tile([P, NT], F32, tag="gam")
            nc.scalar.dma_start(gam[:], gamma_d)
            for t in (2, 3):
                nc.scalar.dma_start(xt8[:, t, :], x_d[t])
            wqk = wp.tile([P, NT, C], FP8, tag="wqk")
            nc.scalar.dma_start(wqk[:], wqk_d)
            wov = wp.tile([P, NT, C], FP8, tag="wov")
            nc.scalar.dma_start(wov[:], wov_d)
            xf = big.tile([P, NT, LQ], BF16, tag="xf")
            nc.sync.dma_start(xf[:], xf_d)
            onesF = small.tile([P, P], F32R, tag="onesF")
            nc.gpsimd.dma_start(onesF[:], onesF_d)
            wg = small.tile([G, NT, P], F32R, tag="wg")
            nc.gpsimd.dma_start(wg[:], wg_d.rearrange("g (t p) -> g t p", p=P))
            vg = small.tile([G, NT, P], F32R, tag="vg")
            nc.gpsimd.dma_start(vg[:], vg_d.rearrange("g (t p) -> g t p", p=P))
            hqk = small.tile([P, NT], F32, tag="hqk")
            nc.gpsimd.dma_start(hqk[:], hqk_d)
            hov = small.tile([P, NT], F32, tag="hov")
            nc.gpsimd.dma_start(hov[:], hov_d)

            # preload the exp table set (Copy/Square/Identity/Exp all live in
            # it -> this is the only ACT table load, during DMA wait)
            shf = small.tile([P, 1], F32, tag="shf")
            nc.vector.memset(shf[:], ES_SHIFT)

            epst = small.tile([G, 1], F32, tag="eps")
            nc.vector.memset(epst[:], EPS)
            dum = tmp.tile([P, 1], F32, tag="dum")
            nc.scalar.activation(dum[:], shf[:], mybir.ActivationFunctionType.Exp,
                                 bias=shf[:])

            # ---- groupnorm stats, per tile: ACT does cols [0,LH) via
            # Copy/Square accum (scaled 1/L), DVE does [LH,L) via bn_stats.
            # The halves meet in the group matmul: gps = pg.m2a + pgh.m2d
            # with pgh = pg * (L-LH)/L. ----
            m2a = small.tile([P, NT, 2], F32, tag="m2a")
            m2d = small.tile([P, NT, 2], F32, tag="m2d")
            ajunk = tmp.tile([P, LH], BF16, tag="ajunk")
            for t in (0, 2, 1, 3):
                nc.scalar.activation(ajunk[:], xt8[:, t, 0:LH],
                                     mybir.ActivationFunctionType.Copy,
                                     scale=1.0 / L,
                                     accum_out=m2a[:, t, 0:1])
                nc.scalar.activation(ajunk[:], xt8[:, t, 0:LH],
                                     mybir.ActivationFunctionType.Square,
                                     scale=1.0 / (L ** 0.5),
                                     accum_out=m2a[:, t, 1:2])
            for t in (0, 2, 1, 3):
                st = tmp.tile([P, len(DCH), 6], F32, tag="bnst")
                off = LH
                for s, w in enumerate(DCH):
                    nc.vector.bn_stats(st[:, s, :], xt8[:, t, off:off + w])
                    off += w
                mv = tmp.tile([P, 2], F32, tag="bnmv")
                nc.vector.bn_aggr(mv[:], st[:])
                # m2d = [mean_d, var_d + mean_d^2] = [mean_d, E_d[x^2]]
                msq = tmp.tile([P, 1], F32, tag="msq")
                nc.vector.tensor_mul(msq[:], mv[:, 0:1], mv[:, 0:1])
                nc.vector.tensor_copy(m2d[:, t, 0:1], mv[:, 0:1])
                nc.vector.tensor_add(m2d[:, t, 1:2], mv[:, 1:2], msq[:])
            gps = ps.tile([G, 2], F32, tag="mm")
            for t in range(NT):
                nc.tensor.matmul(gps[:], pg[:, t, :], m2a[:, t, :],
                                 start=(t == 0), stop=False)
            for t in range(NT):
                nc.tensor.matmul(gps[:], pgh[:, t, :], m2d[:, t, :],
                                 start=False, stop=(t == NT - 1))
            # group stats -> [mean_g, rstd_g]; rstd via Newton rsqrt on DVE
            gsb = small.tile([G, 2], F32R, tag="gsb")
            nc.vector.tensor_copy(gsb[:, 0:1], gps[:, 0:1])
            vrg = tmp.tile([G, 1], F32, tag="vrg")
            nc.vector.tensor_mul(vrg[:], gsb[:, 0:1].bitcast(F32), gsb[:, 0:1].bitcast(F32))
            nc.vector.tensor_tensor(vrg[:], gps[:, 1:2], vrg[:], AOP.subtract)
            nc.vector.tensor_tensor(vrg[:], vrg[:], epst[:], AOP.add)
            # y0 = bitcast(0x5f3759df - (bitcast_i32(v) >> 1)); 2 Newton steps
            magic = nwt[:, 0:1].bitcast(I32)
            one_i = nwt[:, 1:2].bitcast(I32)
            c15 = nwt[:, 2:3]
            ch = nwt[:, 3:4]
            yk = tmp.tile([G, 1], F32, tag="yk")
            nc.vector.tensor_tensor(yk[:].bitcast(I32), vrg[:].bitcast(I32),
                                    one_i, AOP.logical_shift_right)
            nc.vector.tensor_tensor(yk[:].bitcast(I32), magic,
                                    yk[:].bitcast(I32), AOP.subtract)
            vh = tmp.tile([G, 1], F32, tag="vh")
            nc.vector.tensor_tensor(vh[:], vrg[:], ch, AOP.mult)
            t1 = tmp.tile([G, 1], F32, tag="t1")
            for _ in range(1):
                nc.vector.tensor_mul(t1[:], yk[:], yk[:])
                nc.vector.tensor_mul(t1[:], t1[:], vh[:])
                nc.vector.tensor_tensor(t1[:], c15, t1[:], AOP.subtract)
                nc.vector.tensor_mul(yk[:], yk[:], t1[:])
            nc.vector.tensor_copy(gsb[:, 1:2], yk[:])
            # broadcast to channels: chsb[p, t, 0:2] = [mean, rstd] per channel
            chsb = small.tile([P, NT, 2], F32, tag="chsb")
            chs = ps.tile([P, 2 * NT], F32, tag="mm")
            for t in range(NT):
                nc.tensor.matmul(chs[:, 2 * t:2 * t + 2], sel[:, t, :], gsb[:],
                                 start=True, stop=True)
            nc.vector.tensor_copy(chsb[:], chs[:])

            # ---- quantize weights with the rstd fold (gamma is host-folded;
            # DVE so the PE unblocks without waiting on the ACT stats tail) ----
            wqk8 = wp.tile([P, NT, C], FP8, tag="wqk8")
            wov8 = wp.tile([P, NT, C], FP8, tag="wov8")
            for t in range(NT):
                nc.vector.tensor_tensor(wqk8[:, t, :], wqk[:, t, :],
                                        chsb[:, t, 1:2].to_broadcast((P, C)),
                                        AOP.mult)
                nc.scalar.activation(wov8[:, t, :], wov[:, t, :],
                                     mybir.ActivationFunctionType.Copy,
                                     scale=chsb[:, t, 1:2])
            # A = rstd*gamma per channel (cout-side fold for qk8)
            A = small.tile([P, NT], F32, tag="A")
            nc.vector.tensor_mul(A[:], chsb[:, :, 1], gam[:])
            As = small.tile([P, NT], F32, tag="As")
            nc.scalar.activation(As[:], A[:], mybir.ActivationFunctionType.Copy,
                                 scale=1.0 / QK_SCALE)

            # ---- qk8[:, i] = fp8(A.(WqkA x + bqkE)) over query cols.
            # The bias-fold matvecs are emitted after the first qps psum so
            # the PE starts the projection as soon as wqk8 lands; the biases
            # (DVE/PE smalls) complete well before the first qk8 ACT. ----
            qk8 = big.tile([P, NT, LQ], FP8, tag="qk8")
            bias_tiles = {}

            def emit_bias():
                st2 = small.tile([G, 2], F32R, tag="st2")
                nc.vector.tensor_mul(st2[:, 0:1], gsb[:, 0:1].bitcast(F32), gsb[:, 1:2].bitcast(F32))
                nc.vector.tensor_copy(st2[:, 1:2], gsb[:, 0:1].bitcast(F32))
                bqkE = small.tile([P, NT], F32, tag="bqkE")
                bovE = small.tile([P, NT], F32, tag="bovE")
                psB = ps.tile([P, 4 * NT], F32, tag="mm")
                for tq in range(NT):
                    nc.tensor.matmul(psB[:, 2 * tq:2 * tq + 2], wg[:, tq, :], st2[:],
                                     start=True, stop=True)
                    nc.tensor.matmul(psB[:, 2 * NT + 2 * tq:2 * NT + 2 * tq + 2],
                                     vg[:, tq, :], st2[:], start=True, stop=True)
                psBv = psB.rearrange("p (c two) -> p c two", two=2)
                nc.vector.tensor_tensor(bqkE[:], hqk[:], psBv[:, 0:NT, 0],
                                        AOP.subtract)
                nc.vector.tensor_tensor(bovE[:], hov[:], psBv[:, NT:2 * NT, 0],
                                        AOP.subtract)
                qkb = small.tile([P, NT], F32, tag="qkb")
                nc.vector.tensor_mul(qkb[:], A[:], bqkE[:])
                bias_tiles["bovE"] = bovE
                bias_tiles["qkb"] = qkb

            for icn in range(NIC):
                for tq in range(NT):
                    qps = ps.tile([P, IC], F32, tag="mm")
                    for u in range(2):
                        nc.tensor.matmul(qps[:], wqk8[:, 2 * u:2 * u + 2, bass.ts(tq, P)],
                                         xt8[:, 2 * u:2 * u + 2, bass.ts(icn, IC)],
                                         start=(u == 0), stop=(u == 1), perf_mode=DR)
                    if not bias_tiles:
                        emit_bias()
                    nc.scalar.activation(qk8[:, tq, bass.ts(icn, IC)], qps[:],
                                         mybir.ActivationFunctionType.Identity,
                                         scale=As[:, tq:tq + 1],
                                         bias=bias_tiles["qkb"][:, tq:tq + 1])

            # ---- z[t][icn] = x_residual + bovE on GpSimd (idle here) ----
            zall = {}
            for icn in range(NIC):
                for t in range(NT):
                    z = zp.tile([P, IC], F32, tag="zp", name=f"z{icn}_{t}")
                    nc.gpsimd.tensor_tensor(z[:], xf[:, t, bass.ts(icn, IC)],
                                            bias_tiles["bovE"][:, t:t + 1].to_broadcast((P, IC)),
                                            AOP.add)
                    zall[(icn, t)] = z

            # ---- voT projection: voT[j, c] = fp8((WovA x)[c, j]^T)
            # (psum->fp8 casts split DVE/ACT to keep pace with the PE) ----
            vot8 = big.tile([P, NJ, C], FP8, tag="vot8")
            for j in range(NJ):
                vps = ps.tile([P, C], F32, tag="mm")
                for u in range(2):
                    nc.tensor.matmul(vps[:], xt8[:, 2 * u:2 * u + 2, bass.ts(j, P)],
                                     wov8[:, 2 * u:2 * u + 2, :],
                                     start=(u == 0), stop=(u == 1), perf_mode=DR)
                if j % 2 == 0:
                    nc.vector.tensor_copy(vot8[:, j, :], vps[:])
                else:
                    nc.scalar.copy(vot8[:, j, :], vps[:])

            # ---- attention per i-chunk ----
            pending_fin = [None]

            def make_finalize(icn, sums, hops):
                def fin():
                    zs = [zall[(icn, t)] for t in range(NT)]
                    last = icn == NIC - 1
                    HW = IC // 2 if last else IC
                    rbc = tmp.tile([P, IC], BF16, tag="rbc", name=f"rbc{icn}")
                    for h in range(IC // HW):
                        hsl = slice(h * HW, (h + 1) * HW)
                        with nc.allow_low_precision(reason="softmax denom bf16"):
                            nc.vector.reciprocal(rbc[:, hsl], sums[:, hsl])
                        for t in range(NT):
                            o = osb.tile([P, HW], F32, tag="osb",
                                         name=f"o{icn}_{t}_{h}")
                            nc.vector.tensor_tensor(o[:], hops[t][:, hsl],
                                                    rbc[:, hsl], AOP.mult)
                            ob = obf.tile([P, HW], BF16, tag="obf",
                                          name=f"ob{icn}_{t}_{h}")
                            eng = nc.gpsimd if t >= 1 else nc.vector
                            eng.tensor_tensor(ob[:], o[:], zs[t][:, hsl],
                                              AOP.add)
                            deng = nc.sync if t < 2 else nc.scalar
                            deng.dma_start(
                                out_d[bass.ts(t, P),
                                      icn * IC + h * HW:icn * IC + (h + 1) * HW],
                                ob[:])
                return fin

            for icn in range(NIC):
                sums = psum1.tile([P, IC], F32, tag="sums", name=f"sums{icn}")
                hops = [pho.tile([P, IC], F32, tag="ho", name=f"ho_{icn}_{t}")
                        for t in range(NT)]
                # softmax denominator: es accumulated off the PE (DVE takes
                # even j, GpSimd odd j), partition-reduced by one f32r
                # matmul pair at the end of the chunk
                accv = accp.tile([P, IC], F32R, tag="accv", name=f"accv{icn}")
                accg = accp.tile([P, IC], F32R, tag="accg", name=f"accg{icn}")
                espairs = [None] * NPAIR

                def consume(u, hops=hops, espairs=espairs):
                    es = espairs[u]
                    for t in range(NT):
                        nc.tensor.matmul(hops[t][:],
                                         vot8[:, 2 * u:2 * u + 2, bass.ts(t, P)],
                                         es[:, 0:2, :],
                                         start=(u == 0), stop=(u == NPAIR - 1),
                                         perf_mode=DR)
                    espairs[u] = None

                escur = [None]
                for j in range(NJ):
                    u, par = divmod(j, 2)
                    if j == 2 and pending_fin[0] is not None:
                        pending_fin[0]()
                        pending_fin[0] = None
                    sps = ps.tile([P, IC], F32, tag="mm", name=f"sps{icn}_{j}")
                    for uu in range(2):
                        nc.tensor.matmul(sps[:], xt8[:, 2 * uu:2 * uu + 2, bass.ts(j, P)],
                                         qk8[:, 2 * uu:2 * uu + 2, bass.ts(icn, IC)],
                                         start=(uu == 0), stop=(uu == 1), perf_mode=DR)
                    if par == 0:
                        escur[0] = est.tile([P, 2, IC], FP8, tag="est",
                                            name=f"es{icn}_{u}")
                    nc.scalar.activation(escur[0][:, par, :], sps[:],
                                         mybir.ActivationFunctionType.Exp,
                                         bias=shf[:])
                    eng, acc = (nc.vector, accv) if par == 0 else (nc.gpsimd, accg)
                    if j < 2:
                        eng.tensor_copy(acc[:], escur[0][:, par, :])
                    else:
                        eng.tensor_tensor(acc[:], acc[:], escur[0][:, par, :],
                                          AOP.add)
                    if par == 1:
                        espairs[u] = escur[0]
                        if u >= PD:
                            consume(u - PD)
                for u in range(NPAIR - PD, NPAIR):
                    consume(u)
                nc.tensor.matmul(sums[:], onesF[:], accv[:], start=True,
                                 stop=False)
                nc.tensor.matmul(sums[:], onesF[:], accg[:], start=False,
                                 stop=True)
                pending_fin[0] = make_finalize(icn, sums, hops)
            pending_fin[0]()

    nc.compile()
    return nc


def _prep(inputs):
    s = float(C) ** -0.5
    wq = np.asarray(inputs["wq"], np.float64)
    wk = np.asarray(inputs["wk"], np.float64)
    wv = np.asarray(inputs["wv"], np.float64)
    wo = np.asarray(inputs["wo"], np.float64)
    bq = np.asarray(inputs["bq"], np.float64)
    bv = np.asarray(inputs["bv"], np.float64)
    bo = np.asarray(inputs["bo"], np.float64)
    gamma = np.asarray(inputs["gamma"], np.float64)
    beta = np.asarray(inputs["beta"], np.float64)
    Wqk = (wk.T @ wq).T * s      # lhsT layout [c_in, c_out]
    Wov = (wo @ wv).T            # [c_in, c_out]
    bqkv = (wk.T @ bq) * s
    bovv = wo @ bv + bo
    GS = C // G
    WgT = (Wqk * gamma[:, None]).reshape(G, GS, C).sum(axis=1)
    VgT = (Wov * gamma[:, None]).reshape(G, GS, C).sum(axis=1)
    pg = ((np.arange(C)[:, None] // GS == np.arange(G)[None, :])
          .astype(np.float32) / GS)
    dve_frac = 1.0 - LH / L
    # partition-major weight layouts: [p, t, c] flattened to [P, NT*C]
    wqkb = np.clip(Wqk * gamma[:, None] * QK_SCALE, -448, 448).astype(NPF8)
    wovb = np.clip(Wov * gamma[:, None] * OV_SCALE, -448, 448).astype(NPF8)
    nwt = np.zeros(4, np.float32)
    nwt_u = nwt.view(np.uint32)
    nwt_u[0] = 0x5F3759DF
    nwt_u[1] = 1
    nwt[2] = 1.5
    nwt[3] = 0.5
    consts = {
        "wqkT": np.ascontiguousarray(
            wqkb.reshape(NT, P, C).transpose(1, 0, 2).reshape(P, NT * C)),
        "wovT": np.ascontiguousarray(
            wovb.reshape(NT, P, C).transpose(1, 0, 2).reshape(P, NT * C)),
        "wgT": np.ascontiguousarray(WgT, np.float32),
        "vgT": np.ascontiguousarray(VgT, np.float32),
        "hqk": np.ascontiguousarray((Wqk.T @ beta + bqkv).astype(np.float32).reshape(NT, P).T),
        "hov": np.ascontiguousarray((Wov.T @ beta + bovv).astype(np.float32).reshape(NT, P).T),
        "gamma": np.ascontiguousarray(
            np.asarray(inputs["gamma"], np.float32).reshape(NT, P).T),
        "pg": np.ascontiguousarray(pg.reshape(NT, P, G).transpose(1, 0, 2).reshape(P, NT * G)),
        "pgh": np.ascontiguousarray((pg * dve_frac).reshape(NT, P, G).transpose(1, 0, 2).reshape(P, NT * G)),
        "sel": np.ascontiguousarray(
            (np.arange(G)[:, None] == np.arange(C)[None, :] // GS)
            .astype(np.float32)),
        "nwt": np.ascontiguousarray(np.tile(nwt.reshape(1, 4), (G, 1))),
        "onesF": np.ones((P, P), np.float32),
    }
    return consts


LAST_RESULTS = None


def kernel(**inputs) -> np.ndarray:
    global LAST_RESULTS
    if "nc" not in _CACHE:
        _CACHE["nc"] = _build()
    nc = _CACHE["nc"]
    consts = _prep(inputs)
    x = np.asarray(inputs["x"], np.float32)
    xb = x.reshape(B, C, L)
    in_maps = []
    for core in range(NCORES):
        b, chunk = divmod(core, 4)
        xr = np.roll(xb[b], -LQ * chunk, axis=1)
        # x8: [t][p][l] partition-major per tile (4KB DRAM rows)
        x8 = np.ascontiguousarray(xr.reshape(NT, P, L)).astype(NPF8)
        # xf: [p][t*LQ] partition-major (16KB rows)
        xf = np.ascontiguousarray(
            xr[:, :LQ].reshape(NT, P, LQ).transpose(1, 0, 2)
            .reshape(P, NT * LQ)).astype(ml_dtypes.bfloat16)
        in_maps.append({"x8": x8, "xf": xf, **consts})
    res = bass_utils.run_bass_kernel_spmd(nc, in_maps, core_ids=list(range(NCORES)))
    LAST_RESULTS = res
    out = np.empty((B, C, L), np.float32)
    for core in range(NCORES):
        b, chunk = divmod(core, 4)
        out[b][:, LQ * chunk:LQ * (chunk + 1)] = \
            np.asarray(res.results[core]["out"], np.float32)
    return out.reshape(B, C, D, H, W)


# revision 32
# speedup vs baseline: 1.0174x; 1.0174x over previous
"""AttnBlock (GroupNorm + spatial self-attention + residual) on 8 trn2 NeuronCores.

Sharding: 8 cores = 2 batches x 4 query-chunks of 1024 spatial positions.
Each core receives x[b] rolled so its query range is columns [0, 1024); all
cores run one identical SPMD program.

Host-side algebra (exact up to dropped softmax-invariant terms):
  scores^T[j,i] = hn[:,j] . (Wqk hn[:,i] + bqk)   with Wqk = C^-1/2 wk^T wq,
    bqk = C^-1/2 wk^T bq  (the bk term is constant over j -> softmax-invariant)
  out = x + sum_j softmax_j . (Wov hn[:,j]) + bov  with Wov = wo wv,
    bov = wo bv + bo      (softmax rows sum to 1 -> bias moves outside)

Device-side GroupNorm folding: hn = A.x + B per channel; the A-scales fold
into runtime-quantized fp8 weights, B-terms into runtime biases. All five
big matmul groups run fp8e4m3 DoubleRow (2 MACs/PE/cycle). exp(s-5) keeps
es in fp8 range (softmax-invariant shift). Residual stays f32 via xf.

Startup-latency engineering: partition-major DRAM layouts give 4-16KB DMA
packets; groupnorm stats are split per-tile between ACT (first half,
Copy/Square accum) and DVE (second half, bn_stats), combined in the group
reduction matmul with a half-weighted second accumulation; rstd uses a
Newton rsqrt on DVE int/float ALU so the only ACT functions ever used are
{Copy, Square, Identity, Exp} = one table set, preloaded during DMA wait.
"""

import ml_dtypes
import numpy as np

import concourse.bass as bass
import concourse.tile as tile
from concourse import bacc, mybir
from concourse import bass_utils

F32 = mybir.dt.float32
F32R = mybir.dt.float32r
BF16 = mybir.dt.bfloat16
FP8 = mybir.dt.float8e4
I32 = mybir.dt.int32
DR = mybir.MatmulPerfMode.DoubleRow
AOP = mybir.AluOpType

NPF8 = ml_dtypes.float8_e4m3fn

B, C, D, H, W = 2, 512, 4, 32, 32
L = D * H * W            # 4096
G = 32                   # groupnorm groups
EPS = 1e-6
P = 128
NT = C // P              # 4 channel tiles
NJ = L // P              # 32 key tiles
IC = 512                 # i-chunk width
LQ = 1024                # query cols per core
NIC = LQ // IC           # 2 i-chunks
NCORES = 8
NPAIR = NJ // 2          # 16 key-tile pairs per i-chunk
PD = 3                   # attention software-pipeline depth (es pairs ahead)

LH = 1536                # stats split: ACT does cols [0,LH), DVE does [LH,L)
DCH = [512, 512, 512, 512, 512]   # DVE bn_stats chunk widths (sum = L-LH)

QK_SCALE = 256.0         # host prescale on Wqk (fp8 subnormal avoidance)
OV_SCALE = 1.0           # no Wov prescale -> softmax denom needs no unfold
ES_SHIFT = -5.0          # softmax-invariant logit shift (max logit ~9.2)

_CACHE = {}


def _build():
    nc = bacc.Bacc(trn_type="TRN2", target_bir_lowering=False, debug=False,
                   num_devices=NCORES)
    x_d = nc.dram_tensor("x8", [NT, P, L], FP8, kind="ExternalInput").ap()
    xf_d = nc.dram_tensor("xf", [P, NT * LQ], BF16, kind="ExternalInput").ap()
    wqk_d = nc.dram_tensor("wqkT", [P, NT * C], FP8, kind="ExternalInput").ap()
    wov_d = nc.dram_tensor("wovT", [P, NT * C], FP8, kind="ExternalInput").ap()
    pg_d = nc.dram_tensor("pg", [C, G], F32, kind="ExternalInput").ap()
    pgh_d = nc.dram_tensor("pgh", [C, G], F32, kind="ExternalInput").ap()
    sel_d = nc.dram_tensor("sel", [G, C], F32R, kind="ExternalInput").ap()
    nwt_d = nc.dram_tensor("nwt", [G, 4], F32, kind="ExternalInput").ap()
    onesF_d = nc.dram_tensor("onesF", [P, P], F32R, kind="ExternalInput").ap()
    gamma_d = nc.dram_tensor("gamma", [C], F32, kind="ExternalInput").ap()
    wg_d = nc.dram_tensor("wgT", [G, C], F32R, kind="ExternalInput").ap()
    vg_d = nc.dram_tensor("vgT", [G, C], F32R, kind="ExternalInput").ap()
    hqk_d = nc.dram_tensor("hqk", [C], F32, kind="ExternalInput").ap()
    hov_d = nc.dram_tensor("hov", [C], F32, kind="ExternalInput").ap()
    out_d = nc.dram_tensor("out", [C, LQ], BF16, kind="ExternalOutput").ap()

    with tile.TileContext(nc) as tc:
        with (
            tc.tile_pool(name="big", bufs=1) as big,
            tc.tile_pool(name="wp", bufs=1) as wp,
            tc.tile_pool(name="small", bufs=1) as small,
            tc.tile_pool(name="est", bufs=PD + 4) as est,
            tc.tile_pool(name="osb", bufs=3) as osb,
            tc.tile_pool(name="obf", bufs=4) as obf,
            tc.tile_pool(name="hcp", bufs=4) as hcp,
            tc.tile_pool(name="accp", bufs=4) as accp,
            tc.tile_pool(name="zp", bufs=8) as zp,
            tc.tile_pool(name="tmp", bufs=4) as tmp,
            tc.tile_pool(name="ps", bufs=3, space="PSUM") as ps,
            tc.tile_pool(name="pho", bufs=4, space="PSUM") as pho,
            tc.tile_pool(name="psum1", bufs=1, space="PSUM") as psum1,
        ):
            # ---- DMA: x8 tiles split across the two HWDGE rings
            # (partition-major: 4KB rows per tile -> big packets) ----
            xt8 = big.tile([P, NT, L], FP8, tag="xt8")
            for t in (0, 1):
                nc.sync.dma_start(xt8[:, t, :], x_d[t])
            for t in (2, 3):
                nc.scalar.dma_start(xt8[:, t, :], x_d[t])
            wqk = wp.tile([P, NT, C], FP8, tag="wqk")
            nc.scalar.dma_start(wqk[:], wqk_d)
            wov = wp.tile([P, NT, C], FP8, tag="wov")
            nc.scalar.dma_start(wov[:], wov_d)
            xf = big.tile([P, NT, LQ], BF16, tag="xf")
            nc.sync.dma_start(xf[:], xf_d)
            pg = small.tile([P, NT, G], F32, tag="pg")
            nc.gpsimd.dma_start(pg[:], pg_d.rearrange("(t p) g -> p t g", p=P))
            pgh = small.tile([P, NT, G], F32, tag="pgh")
            nc.gpsimd.dma_start(pgh[:], pgh_d.rearrange("(t p) g -> p t g", p=P))
            sel = small.tile([G, NT, P], F32R, tag="sel")
            nc.gpsimd.dma_start(sel[:], sel_d.rearrange("g (t p) -> g t p", p=P))
            nwt = small.tile([G, 4], F32, tag="nwt")
            nc.gpsimd.dma_start(nwt[:], nwt_d)
            onesF = small.tile([P, P], F32R, tag="onesF")
            nc.gpsimd.dma_start(onesF[:], onesF_d)
            gam = small.tile([P, NT], F32, tag="gam")
            nc.gpsimd.dma_start(gam[:], gamma_d.rearrange("(t p) -> p t", p=P))
            wg = small.tile([G, NT, P], F32R, tag="wg")
            nc.gpsimd.dma_start(wg[:], wg_d.rearrange("g (t p) -> g t p", p=P))
            vg = small.tile([G, NT, P], F32R, tag="vg")
            nc.gpsimd.dma_start(vg[:], vg_d.rearrange("g (t p) -> g t p", p=P))
            hqk = small.tile([P, NT], F32, tag="hqk")
            nc.gpsimd.dma_start(hqk[:], hqk_d.rearrange("(t p) -> p t", p=P))
            hov = small.tile([P, NT], F32, tag="hov")
            nc.gpsimd.dma_start(hov[:], hov_d.rearrange("(t p) -> p t", p=P))

            # preload the exp table set (Copy/Square/Identity/Exp all live in
            # it -> this is the only ACT table load, during DMA wait)
            shf = small.tile([P, 1], F32, tag="shf")
            nc.vector.memset(shf[:], ES_SHIFT)

            epst = small.tile([G, 1], F32, tag="eps")
            nc.vector.memset(epst[:], EPS)
            dum = tmp.tile([P, 1], F32, tag="dum")
            nc.scalar.activation(dum[:], shf[:], mybir.ActivationFunctionType.Exp,
                                 bias=shf[:])

            # ---- groupnorm stats, per tile: ACT does cols [0,LH) via
            # Copy/Square accum (scaled 1/L), DVE does [LH,L) via bn_stats.
            # The halves meet in the group matmul: gps = pg.m2a + pgh.m2d
            # with pgh = pg * (L-LH)/L. ----
            m2a = small.tile([P, NT, 2], F32, tag="m2a")
            m2d = small.tile([P, NT, 2], F32, tag="m2d")
            ajunk = tmp.tile([P, LH], BF16, tag="ajunk")
            for t in (0, 2, 1, 3):
                nc.scalar.activation(ajunk[:], xt8[:, t, 0:LH],
                                     mybir.ActivationFunctionType.Copy,
                                     scale=1.0 / L,
                                     accum_out=m2a[:, t, 0:1])
                nc.scalar.activation(ajunk[:], xt8[:, t, 0:LH],
                                     mybir.ActivationFunctionType.Square,
                                     scale=1.0 / (L ** 0.5),
                                     accum_out=m2a[:, t, 1:2])
            for t in (0, 2, 1, 3):
                st = tmp.tile([P, len(DCH), 6], F32, tag="bnst")
                off = LH
                for s, w in enumerate(DCH):
                    nc.vector.bn_stats(st[:, s, :], xt8[:, t, off:off + w])
                    off += w
                mv = tmp.tile([P, 2], F32, tag="bnmv")
                nc.vector.bn_aggr(mv[:], st[:])
                # m2d = [mean_d, var_d + mean_d^2] = [mean_d, E_d[x^2]]
                msq = tmp.tile([P, 1], F32, tag="msq")
                nc.vector.tensor_mul(msq[:], mv[:, 0:1], mv[:, 0:1])
                nc.vector.tensor_copy(m2d[:, t, 0:1], mv[:, 0:1])
                nc.vector.tensor_add(m2d[:, t, 1:2], mv[:, 1:2], msq[:])
            gps = ps.tile([G, 2], F32, tag="mm")
            for t in range(NT):
                nc.tensor.matmul(gps[:], pg[:, t, :], m2a[:, t, :],
                                 start=(t == 0), stop=False)
            for t in range(NT):
                nc.tensor.matmul(gps[:], pgh[:, t, :], m2d[:, t, :],
                                 start=False, stop=(t == NT - 1))
            # group stats -> [mean_g, rstd_g]; rstd via Newton rsqrt on DVE
            gsb = small.tile([G, 2], F32R, tag="gsb")
            nc.vector.tensor_copy(gsb[:, 0:1], gps[:, 0:1])
            vrg = tmp.tile([G, 1], F32, tag="vrg")
            nc.vector.tensor_mul(vrg[:], gsb[:, 0:1].bitcast(F32), gsb[:, 0:1].bitcast(F32))
            nc.vector.tensor_tensor(vrg[:], gps[:, 1:2], vrg[:], AOP.subtract)
            nc.vector.tensor_tensor(vrg[:], vrg[:], epst[:], AOP.add)
            # y0 = bitcast(0x5f3759df - (bitcast_i32(v) >> 1)); 2 Newton steps
            magic = nwt[:, 0:1].bitcast(I32)
            one_i = nwt[:, 1:2].bitcast(I32)
            c15 = nwt[:, 2:3]
            ch = nwt[:, 3:4]
            yk = tmp.tile([G, 1], F32, tag="yk")
            nc.vector.tensor_tensor(yk[:].bitcast(I32), vrg[:].bitcast(I32),
                                    one_i, AOP.logical_shift_right)
            nc.vector.tensor_tensor(yk[:].bitcast(I32), magic,
                                    yk[:].bitcast(I32), AOP.subtract)
            vh = tmp.tile([G, 1], F32, tag="vh")
            nc.vector.tensor_tensor(vh[:], vrg[:], ch, AOP.mult)
            t1 = tmp.tile([G, 1], F32, tag="t1")
            for _ in range(1):
                nc.vector.tensor_mul(t1[:], yk[:], yk[:])
                nc.vector.tensor_mul(t1[:], t1[:], vh[:])
                nc.vector.tensor_tensor(t1[:], c15, t1[:], AOP.subtract)
                nc.vector.tensor_mul(yk[:], yk[:], t1[:])
            nc.vector.tensor_copy(gsb[:, 1:2], yk[:])
            # broadcast to channels: chsb[p, t, 0:2] = [mean, rstd] per channel
            chsb = small.tile([P, NT, 2], F32, tag="chsb")
            chs = ps.tile([P, 2 * NT], F32, tag="mm")
            for t in range(NT):
                nc.tensor.matmul(chs[:, 2 * t:2 * t + 2], sel[:, t, :], gsb[:],
                                 start=True, stop=True)
            nc.vector.tensor_copy(chsb[:], chs[:])

            # ---- quantize weights with the rstd fold (gamma is host-folded;
            # DVE so the PE unblocks without waiting on the ACT stats tail) ----
            wqk8 = wp.tile([P, NT, C], FP8, tag="wqk8")
            wov8 = wp.tile([P, NT, C], FP8, tag="wov8")
            for t in range(NT):
                nc.vector.tensor_tensor(wqk8[:, t, :], wqk[:, t, :],
                                        chsb[:, t, 1:2].to_broadcast((P, C)),
                                        AOP.mult)
                nc.scalar.activation(wov8[:, t, :], wov[:, t, :],
                                     mybir.ActivationFunctionType.Copy,
                                     scale=chsb[:, t, 1:2])
            # A = rstd*gamma per channel (cout-side fold for qk8)
            A = small.tile([P, NT], F32, tag="A")
            nc.vector.tensor_mul(A[:], chsb[:, :, 1], gam[:])
            As = small.tile([P, NT], F32, tag="As")
            nc.scalar.activation(As[:], A[:], mybir.ActivationFunctionType.Copy,
                                 scale=1.0 / QK_SCALE)

            # ---- qk8[:, i] = fp8(A.(WqkA x + bqkE)) over query cols.
            # The bias-fold matvecs are emitted after the first qps psum so
            # the PE starts the projection as soon as wqk8 lands; the biases
            # (DVE/PE smalls) complete well before the first qk8 ACT. ----
            qk8 = big.tile([P, NT, LQ], FP8, tag="qk8")
            bias_tiles = {}

            def emit_bias():
                st2 = small.tile([G, 2], F32R, tag="st2")
                nc.vector.tensor_mul(st2[:, 0:1], gsb[:, 0:1].bitcast(F32), gsb[:, 1:2].bitcast(F32))
                nc.vector.tensor_copy(st2[:, 1:2], gsb[:, 0:1].bitcast(F32))
                bqkE = small.tile([P, NT], F32, tag="bqkE")
                bovE = small.tile([P, NT], F32, tag="bovE")
                psB = ps.tile([P, 4 * NT], F32, tag="mm")
                for tq in range(NT):
                    nc.tensor.matmul(psB[:, 2 * tq:2 * tq + 2], wg[:, tq, :], st2[:],
                                     start=True, stop=True)
                    nc.tensor.matmul(psB[:, 2 * NT + 2 * tq:2 * NT + 2 * tq + 2],
                                     vg[:, tq, :], st2[:], start=True, stop=True)
                psBv = psB.rearrange("p (c two) -> p c two", two=2)
                nc.vector.tensor_tensor(bqkE[:], hqk[:], psBv[:, 0:NT, 0],
                                        AOP.subtract)
                nc.vector.tensor_tensor(bovE[:], hov[:], psBv[:, NT:2 * NT, 0],
                                        AOP.subtract)
                qkb = small.tile([P, NT], F32, tag="qkb")
                nc.vector.tensor_mul(qkb[:], A[:], bqkE[:])
                bias_tiles["bovE"] = bovE
                bias_tiles["qkb"] = qkb

            for icn in range(NIC):
                for tq in range(NT):
                    qps = ps.tile([P, IC], F32, tag="mm")
                    for u in range(2):
                        nc.tensor.matmul(qps[:], wqk8[:, 2 * u:2 * u + 2, bass.ts(tq, P)],
                                         xt8[:, 2 * u:2 * u + 2, bass.ts(icn, IC)],
                                         start=(u == 0), stop=(u == 1), perf_mode=DR)
                    if not bias_tiles:
                        emit_bias()
                    nc.scalar.activation(qk8[:, tq, bass.ts(icn, IC)], qps[:],
                                         mybir.ActivationFunctionType.Identity,
                                         scale=As[:, tq:tq + 1],
                                         bias=bias_tiles["qkb"][:, tq:tq + 1])

            # ---- z[t][icn] = x_residual + bovE on GpSimd (idle here) ----
            zall = {}
            for icn in range(NIC):
                for t in range(NT):
                    z = zp.tile([P, IC], F32, tag="zp", name=f"z{icn}_{t}")
                    nc.gpsimd.tensor_tensor(z[:], xf[:, t, bass.ts(icn, IC)],
                                            bias_tiles["bovE"][:, t:t + 1].to_broadcast((P, IC)),
                                            AOP.add)
                    zall[(icn, t)] = z

            # ---- voT projection: voT[j, c] = fp8((WovA x)[c, j]^T)
            # (psum->fp8 casts split DVE/ACT to keep pace with the PE) ----
            vot8 = big.tile([P, NJ, C], FP8, tag="vot8")
            for j in range(NJ):
                vps = ps.tile([P, C], F32, tag="mm")
                for u in range(2):
                    nc.tensor.matmul(vps[:], xt8[:, 2 * u:2 * u + 2, bass.ts(j, P)],
                                     wov8[:, 2 * u:2 * u + 2, :],
                                     start=(u == 0), stop=(u == 1), perf_mode=DR)
                if j % 2 == 0:
                    nc.vector.tensor_copy(vot8[:, j, :], vps[:])
                else:
                    nc.scalar.copy(vot8[:, j, :], vps[:])

            # ---- attention per i-chunk ----
            pending_fin = [None]

            def make_finalize(icn, sums, hops):
                def fin():
                    zs = [zall[(icn, t)] for t in range(NT)]
                    last = icn == NIC - 1
                    # non-last chunk: drain hops psum banks to SBUF right away
                    # so the next chunk's first consume isn't blocked on the
                    # whole recip+mult chain
                    hsrc = hops
                    if not last:
                        hsb = []
                        for t in range(NT):
                            hc = hcp.tile([P, IC], F32, tag="hcp",
                                          name=f"hc{icn}_{t}")
                            nc.vector.tensor_copy(hc[:], hops[t][:])
                            hsb.append(hc)
                        hsrc = hsb
                    HW = IC // 2 if last else IC
                    rbc = tmp.tile([P, IC], BF16, tag="rbc", name=f"rbc{icn}")
                    for h in range(IC // HW):
                        hsl = slice(h * HW, (h + 1) * HW)
                        with nc.allow_low_precision(reason="softmax denom bf16"):
                            nc.vector.reciprocal(rbc[:, hsl], sums[:, hsl])
                        for t in range(NT):
                            o = osb.tile([P, HW], F32, tag="osb",
                                         name=f"o{icn}_{t}_{h}")
                            nc.vector.tensor_tensor(o[:], hsrc[t][:, hsl],
                                                    rbc[:, hsl], AOP.mult)
                            ob = obf.tile([P, HW], BF16, tag="obf",
                                          name=f"ob{icn}_{t}_{h}")
                            eng = nc.gpsimd if t >= 1 else nc.vector
                            eng.tensor_tensor(ob[:], o[:], zs[t][:, hsl],
                                              AOP.add)
                            deng = nc.sync if t < 2 else nc.scalar
                            deng.dma_start(
                                out_d[bass.ts(t, P),
                                      icn * IC + h * HW:icn * IC + (h + 1) * HW],
                                ob[:])
                return fin

            for icn in range(NIC):
                sums = psum1.tile([P, IC], F32, tag="sums", name=f"sums{icn}")
                hops = [pho.tile([P, IC], F32, tag="ho", name=f"ho_{icn}_{t}")
                        for t in range(NT)]
                # softmax denominator: es accumulated off the PE (DVE takes
                # even j, GpSimd odd j), partition-reduced by one f32r
                # matmul pair at the end of the chunk
                accv = accp.tile([P, IC], F32R, tag="accv", name=f"accv{icn}")
                accg = accp.tile([P, IC], F32R, tag="accg", name=f"accg{icn}")
                espairs = [None] * NPAIR

                def consume(u, hops=hops, espairs=espairs):
                    es = espairs[u]
                    for t in range(NT):
                        nc.tensor.matmul(hops[t][:],
                                         vot8[:, 2 * u:2 * u + 2, bass.ts(t, P)],
                                         es[:, 0:2, :],
                                         start=(u == 0), stop=(u == NPAIR - 1),
                                         perf_mode=DR)
                    espairs[u] = None

                escur = [None]
                for j in range(NJ):
                    u, par = divmod(j, 2)
                    if j == 2 and pending_fin[0] is not None:
                        pending_fin[0]()
                        pending_fin[0] = None
                    sps = ps.tile([P, IC], F32, tag="mm", name=f"sps{icn}_{j}")
                    for uu in range(2):
                        nc.tensor.matmul(sps[:], xt8[:, 2 * uu:2 * uu + 2, bass.ts(j, P)],
                                         qk8[:, 2 * uu:2 * uu + 2, bass.ts(icn, IC)],
                                         start=(uu == 0), stop=(uu == 1), perf_mode=DR)
                    if par == 0:
                        escur[0] = est.tile([P, 2, IC], FP8, tag="est",
                                            name=f"es{icn}_{u}")
                    nc.scalar.activation(escur[0][:, par, :], sps[:],
                                         mybir.ActivationFunctionType.Exp,
                                         bias=shf[:])
                    eng, acc = (nc.vector, accv) if par == 0 else (nc.gpsimd, accg)
                    if j < 2:
                        eng.tensor_copy(acc[:], escur[0][:, par, :])
                    else:
                        eng.tensor_tensor(acc[:], acc[:], escur[0][:, par, :],
                                          AOP.add)
                    if par == 1:
                        espairs[u] = escur[0]
                        if u >= PD:
                            consume(u - PD)
                for u in range(NPAIR - PD, NPAIR):
                    consume(u)
                nc.tensor.matmul(sums[:], onesF[:], accv[:], start=True,
                                 stop=False)
                nc.tensor.matmul(sums[:], onesF[:], accg[:], start=False,
                                 stop=True)
                pending_fin[0] = make_finalize(icn, sums, hops)
            pending_fin[0]()

    nc.compile()
    return nc


def _prep(inputs):
    s = float(C) ** -0.5
    wq = np.asarray(inputs["wq"], np.float64)
    wk = np.asarray(inputs["wk"], np.float64)
    wv = np.asarray(inputs["wv"], np.float64)
    wo = np.asarray(inputs["wo"], np.float64)
    bq = np.asarray(inputs["bq"], np.float64)
    bv = np.asarray(inputs["bv"], np.float64)
    bo = np.asarray(inputs["bo"], np.float64)
    gamma = np.asarray(inputs["gamma"], np.float64)
    beta = np.asarray(inputs["beta"], np.float64)
    Wqk = (wk.T @ wq).T * s      # lhsT layout [c_in, c_out]
    Wov = (wo @ wv).T            # [c_in, c_out]
    bqkv = (wk.T @ bq) * s
    bovv = wo @ bv + bo
    GS = C // G
    WgT = (Wqk * gamma[:, None]).reshape(G, GS, C).sum(axis=1)
    VgT = (Wov * gamma[:, None]).reshape(G, GS, C).sum(axis=1)
    pg = ((np.arange(C)[:, None] // GS == np.arange(G)[None, :])
          .astype(np.float32) / GS)
    dve_frac = 1.0 - LH / L
    # partition-major weight layouts: [p, t, c] flattened to [P, NT*C]
    wqkb = np.clip(Wqk * gamma[:, None] * QK_SCALE, -448, 448).astype(NPF8)
    wovb = np.clip(Wov * gamma[:, None] * OV_SCALE, -448, 448).astype(NPF8)
    nwt = np.zeros(4, np.float32)
    nwt_u = nwt.view(np.uint32)
    nwt_u[0] = 0x5F3759DF
    nwt_u[1] = 1
    nwt[2] = 1.5
    nwt[3] = 0.5
    consts = {
        "wqkT": np.ascontiguousarray(
            wqkb.reshape(NT, P, C).transpose(1, 0, 2).reshape(P, NT * C)),
        "wovT": np.ascontiguousarray(
            wovb.reshape(NT, P, C).transpose(1, 0, 2).reshape(P, NT * C)),
        "wgT": np.ascontiguousarray(WgT, np.float32),
        "vgT": np.ascontiguousarray(VgT, np.float32),
        "hqk": (Wqk.T @ beta + bqkv).astype(np.float32),
        "hov": (Wov.T @ beta + bovv).astype(np.float32),
        "gamma": np.asarray(inputs["gamma"], np.float32),
        "pg": np.ascontiguousarray(pg),
        "pgh": np.ascontiguousarray(pg * dve_frac),
        "sel": np.ascontiguousarray(
            (np.arange(G)[:, None] == np.arange(C)[None, :] // GS)
            .astype(np.float32)),
        "nwt": np.ascontiguousarray(np.tile(nwt.reshape(1, 4), (G, 1))),
        "onesF": np.ones((P, P), np.float32),
    }
    return consts


LAST_RESULTS = None


def kernel(**inputs) -> np.ndarray:
    global LAST_RESULTS
    if "nc" not in _CACHE:
        _CACHE["nc"] = _build()
    nc = _CACHE["nc"]
    consts = _prep(inputs)
    x = np.asarray(inputs["x"], np.float32)
    xb = x.reshape(B, C, L)
    in_maps = []
    for core in range(NCORES):
        b, chunk = divmod(core, 4)
        xr = np.roll(xb[b], -LQ * chunk, axis=1)
        # x8: [t][p][l] partition-major per tile (4KB DRAM rows)
        x8 = np.ascontiguousarray(xr.reshape(NT, P, L)).astype(NPF8)
        # xf: [p][t*LQ] partition-major (16KB rows)
        xf = np.ascontiguousarray(
            xr[:, :LQ].reshape(NT, P, LQ).transpose(1, 0, 2)
            .reshape(P, NT * LQ)).astype(ml_dtypes.bfloat16)
        in_maps.append({"x8": x8, "xf": xf, **consts})
    res = bass_utils.run_bass_kernel_spmd(nc, in_maps, core_ids=list(range(NCORES)))
    LAST_RESULTS = res
    out = np.empty((B, C, L), np.float32)
    for core in range(NCORES):
        b, chunk = divmod(core, 4)
        out[b][:, LQ * chunk:LQ * (chunk + 1)] = \
            np.asarray(res.results[core]["out"], np.float32)
    return out.reshape(B, C, D, H, W)
